# revision 40
# baseline (speedup 1.0000x reference)
"""Multi-head attention (B=4, S=2048, D=1024, H=16, d_k=64) on 8 TRN2 NeuronCores.

Sharding: batch x head-group. Core c handles batch b = c//2 and heads
[8*(c%2), 8*(c%2)+8). Each core computes Q/K/V projections for its 512
output features (column-parallel), attention for its 8 heads, and a
row-parallel partial of the W_o output projection. The host sums the two
partials per batch (the row-parallel unshard) — no collectives needed.

Device schedule (per core): one software-pipelined stream. Attention is
split into 256 groups (4 q-chunks x 4 head-pairs x 16 key-blocks). Per
group the PE runs 2 score matmuls (N=512) and 2 attn@V matmuls; the ACT
engine runs one 1024-free exp per group and is the critical engine
(~280us), so everything else (Q/K/V projections, W_o, softmax
normalization) is emitted as filler inside the attention stream. attn@V
consumption is skewed 2 groups behind scores so the PE never head-waits
on the exp latency. PSUM: 3 rotating 2-bank score slots (also reused by
projection/W_o/normalization inserts) + 2 single-bank attn@V
accumulators (avA/avB of one pair; the skew covers their evacuation, so
the next pair reuses them without stalling). Softmax denominators ride
as a 65th V row; reciprocals are batched per half-q-chunk ([34, 512]
layout, pairs at partitions 0/32) on DVE; normalization and W_o are
deferred so their dependencies are always long ready.
"""

import os
from collections import defaultdict

import numpy as np
import ml_dtypes

import concourse.bacc as bacc
import concourse.mybir as mybir
import concourse.tile as tile
from concourse.bass_utils import run_bass_kernel_spmd

BF16 = mybir.dt.bfloat16
F32 = mybir.dt.float32
EXP = mybir.ActivationFunctionType.Exp

B, S, D = 4, 2048, 1024
H, DK = 16, 64
HPC = 8           # heads per core
FPC = HPC * DK    # 512 features per core
NP = 4            # head pairs per core
NB = 8            # din blocks of 128
NKB = 16          # key blocks of 128
QC = 512          # query chunk
NQC = S // QC     # 4
NG = NKB          # groups (1 key block each) per (qc, m)

_nc_cache = None
last_results = None


def build():
    nc = bacc.Bacc("TRN2", target_bir_lowering=False, debug=False, num_devices=8)

    xq = nc.dram_tensor("xq", [D, S], BF16, kind="ExternalInput").ap()
    xk = nc.dram_tensor("xk", [D, S], BF16, kind="ExternalInput").ap()
    xv = nc.dram_tensor("xv", [D, S], BF16, kind="ExternalInput").ap()
    wq = nc.dram_tensor("wq", [D, FPC], BF16, kind="ExternalInput").ap()
    wk = nc.dram_tensor("wk", [D, FPC], BF16, kind="ExternalInput").ap()
    wv = nc.dram_tensor("wv", [D, FPC], BF16, kind="ExternalInput").ap()
    wo = nc.dram_tensor("wo", [FPC, D], BF16, kind="ExternalInput").ap()
    mask = nc.dram_tensor("mask", [8, 512], BF16, kind="ExternalInput").ap()
    out = nc.dram_tensor("out", [S, D], F32, kind="ExternalOutput").ap()

    with tile.TileContext(nc) as tc:
        with (
            tc.tile_pool(name="wp", bufs=1) as wp,
            tc.tile_pool(name="qkv", bufs=1) as qkv,
            tc.tile_pool(name="xp", bufs=1) as xp,
            tc.tile_pool(name="xvp", bufs=3) as xvp,
            tc.tile_pool(name="ptp", bufs=4) as ptp,
            tc.tile_pool(name="avsb", bufs=2) as avsb,
            tc.tile_pool(name="otp", bufs=2) as otp,
            tc.tile_pool(name="denp", bufs=2) as denp,
            tc.tile_pool(name="recp", bufs=2) as recp,
            tc.tile_pool(name="outp", bufs=1) as outp,
            tc.tile_pool(name="sp", bufs=2, space="PSUM") as sp,
            tc.tile_pool(name="avp", bufs=2, space="PSUM") as avp,
            tc.tile_pool(name="miscp", bufs=2, space="PSUM") as miscp,
        ):
            # ---- static SBUF tensors + input DMAs (priority order) ----
            wq_sb = wp.tile([128, NB, NP, 128], BF16, tag="wq")
            wk_sb = wp.tile([128, NB, NP, 128], BF16, tag="wk")
            wv_sb = wp.tile([128, NB, FPC], BF16, tag="wv")
            wo_sb = wp.tile([128, NP, D], BF16, tag="wo")
            m_sb = wp.tile([8, NP, 128], BF16, tag="mask")
            qt_sb = qkv.tile([128, NP, S], BF16, tag="qt")
            kt_sb = qkv.tile([128, NP, S], BF16, tag="kt")
            v_sb = qkv.tile([128, NKB, HPC, 65], BF16, tag="v")
            xq_sb = xp.tile([128, NB, S], BF16, tag="xq")
            xk_sb = xp.tile([128, NB, S], BF16, tag="xk")

            # DMAs ordered by first consumption time. Each dma_start costs
            # ~600ns serially on the Sync sequencer, so bulk transfers use
            # few, larger descriptors (split only enough to spread queues);
            # the prologue-critical chunks are split 8-way for landing time.
            def dma_w(w_sb, w, m, nsplit=1):
                step = NB // nsplit
                for s in range(nsplit):
                    b0 = s * step
                    nc.sync.dma_start(
                        w_sb[:, b0:b0 + step, m],
                        w[:, m * 128:(m + 1) * 128]
                        .rearrange("(b p) c -> p b c", p=128)[:, b0:b0 + step])

            def dma_x(x_sb, x, c, nsplit=2):
                step = NB // nsplit
                for s in range(nsplit):
                    b0 = s * step
                    nc.sync.dma_start(
                        x_sb[:, b0:b0 + step, c * 512:(c + 1) * 512],
                        x[:, c * 512:(c + 1) * 512]
                        .rearrange("(b p) t -> p b t", p=128)[:, b0:b0 + step])

            xv_ch = {}
            def load_xv_chunk(c, nsplit=4):
                ch = xvp.tile([128, NB, 512], BF16, tag="xv", name=f"xv{c}")
                xv_ch[c] = ch
                step = NB // nsplit
                for s in range(nsplit):
                    b0 = s * step
                    nc.sync.dma_start(
                        ch[:, b0:b0 + step],
                        xv[:, c * 512:(c + 1) * 512]
                        .rearrange("(b p) t -> p b t", p=128)[:, b0:b0 + step])

            nc.sync.dma_start(m_sb[:], mask.rearrange("p (j c) -> p j c", c=128))
            dma_w(wq_sb, wq, 0, nsplit=2)
            dma_x(xq_sb, xq, 0, nsplit=8)
            dma_w(wk_sb, wk, 0, nsplit=2)
            dma_x(xk_sb, xk, 0, nsplit=8)
            for s in range(4):
                nc.sync.dma_start(
                    wv_sb[:, 2 * s:2 * s + 2],
                    wv.rearrange("(b p) c -> p b c", p=128)[:, 2 * s:2 * s + 2])
            load_xv_chunk(0)
            dma_x(xk_sb, xk, 1, nsplit=4)
            load_xv_chunk(1)
            dma_x(xk_sb, xk, 2, nsplit=4)
            dma_x(xk_sb, xk, 3, nsplit=4)
            dma_w(wq_sb, wq, 1)
            dma_w(wk_sb, wk, 1)
            load_xv_chunk(2)
            dma_w(wq_sb, wq, 2)
            dma_w(wk_sb, wk, 2)
            dma_w(wq_sb, wq, 3)
            dma_w(wk_sb, wk, 3)
            dma_x(xq_sb, xq, 1)
            for s in range(2):
                nc.sync.dma_start(
                    wo_sb[:, 2 * s:2 * s + 2],
                    wo.rearrange("(f p) c -> p f c", p=128)[:, 2 * s:2 * s + 2])
            dma_x(xq_sb, xq, 2)
            dma_x(xq_sb, xq, 3)

            nc.vector.memset(v_sb[:, :, :, 64], 1.0)

            # ---- filler tasks: generators yielding one PE matmul per step ----
            # Each task streams its accumulation through a 1-bank misc slot;
            # at most one task is open at a time (miscp bufs=2 lets the next
            # task's matmuls start while the previous slot's copy drains).
            def proj_chunk_task(x_sb, w_sb, dst, m, c):
                """512-token projection chunk: 8 matmuls + evac copy."""
                ps = miscp.tile([128, 512], F32, tag="misc", name="projps")
                lo = c * 512
                for b in range(NB):
                    nc.tensor.matmul(ps[:], w_sb[:, b, m], x_sb[:, b, lo:lo + 512],
                                     start=(b == 0), stop=(b == NB - 1))
                    if b < NB - 1:
                        yield
                nc.vector.tensor_copy(dst[:, m, lo:lo + 512], ps[:])

            def v_chunk_task(tt):
                """V projection for token tile tt: 8 matmuls + evac copy."""
                ps = miscp.tile([128, 512], F32, tag="misc", name="vps")
                ch = xv_ch[tt // 4]
                off = (tt % 4) * 128
                for b in range(NB):
                    nc.tensor.matmul(ps[:], ch[:, b, off:off + 128], wv_sb[:, b],
                                     start=(b == 0), stop=(b == NB - 1))
                    if b < NB - 1:
                        yield
                nc.vector.tensor_copy(
                    v_sb[:, tt, :, 0:64],
                    ps[:].rearrange("p (h c) -> p h c", c=64))

            ot_tiles = {}
            avsb_tiles = {}
            den_tiles = {}
            rec_tiles = {}

            def norm_task(qc, m, pool=None):
                """Normalize pair (qc, m): broadcast 1/den via mask matmul, mul."""
                rec2 = rec_tiles[(qc, m)]
                if m == 0:
                    ot_tiles[qc] = otp.tile([128, NP, QC], BF16, tag="ot", name="ot")
                ot = ot_tiles[qc]
                av_sb = avsb_tiles.pop((qc, m))
                if pool is None:
                    scp = miscp.tile([128, 512], F32, tag="misc", name="scp")
                else:
                    scp = pool.tile([128, 1024], F32, tag="s", name="scp")
                for j in range(4):
                    nc.tensor.matmul(scp[:, 128 * j:128 * (j + 1)], m_sb[:, j],
                                     rec2[:], start=True, stop=True)
                nc.vector.tensor_mul(ot[0:64, m], av_sb[0:64, 0:QC], scp[0:64, 0:QC])
                nc.vector.tensor_mul(ot[64:128, m], av_sb[0:64, QC:2 * QC], scp[64:128, 0:QC])
                return
                yield

            def wo_task(qc, tt, jc):
                """Half of the output projection for token block (qc, tt)."""
                ot = ot_tiles[qc]
                wop = miscp.tile([128, 512], F32, tag="misc", name="wop")
                tsl = slice(tt * 128, (tt + 1) * 128)
                for fb in range(NP):
                    nc.tensor.matmul(
                        wop[:], ot[:, fb, tsl], wo_sb[:, fb, jc * 512:(jc + 1) * 512],
                        start=(fb == 0), stop=(fb == NP - 1))
                    if fb < NP - 1:
                        yield
                if jc == 0:
                    ostage_tiles[(qc, tt)] = outp.tile([128, D], F32, tag="ostage",
                                                       name="ostage")
                ostage = ostage_tiles[(qc, tt)]
                nc.vector.tensor_copy(ostage[:, jc * 512:(jc + 1) * 512], wop[:])
                row = qc * QC + tt * 128
                for h in range(2):
                    lo = jc * 512 + h * 256
                    nc.sync.dma_start(out[row:row + 128, lo:lo + 256],
                                      ostage[:, lo:lo + 256])

            ostage_tiles = {}

            # Task queue: (due_index, generator). Pump keeps deadline order.
            tasks = []
            def add_task(due, gen):
                tasks.append((due, gen))
                tasks.sort(key=lambda t: t[0])

            open_task = None
            open_due = 0

            def pump(i, min_steps):
                """Emit filler matmul-steps: min_steps per index, running
                ahead only to finish overdue tasks (deadline-driven)."""
                nonlocal open_task, open_due
                emitted = 0
                while True:
                    if open_task is None and tasks and (
                            emitted < min_steps or tasks[0][0] <= i + 2):
                        open_due, open_task = tasks.pop(0)
                    if open_task is None:
                        return
                    if emitted >= min_steps and open_due > i:
                        return
                    try:
                        next(open_task)
                    except StopIteration:
                        open_task = None
                    emitted += 1

            # Due = first index whose scores/av READ the produced tile, minus
            # 3 so the pump's overdue-drain finishes the task strictly before
            # the consumer is emitted (PE program order is emission order).
            for tt in range(NKB):
                if tt >= 1:
                    add_task(2 + tt, v_chunk_task(tt))
            for m in range(NP):
                for c in range(4):
                    if not (m == 0 and c == 0):
                        add_task(16 * m + 4 * c - 3,
                                 proj_chunk_task(xk_sb, wk_sb, kt_sb, m, c))
            for m in range(NP):
                for c in range(4):
                    if not (m == 0 and c == 0):
                        add_task(64 * c + 16 * m - 3,
                                 proj_chunk_task(xq_sb, wq_sb, qt_sb, m, c))
            def load_xv_chunk_task(c):
                load_xv_chunk(c)
                return
                yield

            fill_at = defaultdict(list)
            fill_at[3].append(lambda: load_xv_chunk_task(3))
            # normalization / Wo release points: pair p=(qc, m) spans indices
            # [16p, 16p+16). norm of pair p-2 at +10 (reciprocal chain of p-2
            # finished ~5 indices earlier); Wo of q-chunk qc-1 in (qc, m2/m3).
            for qc in range(NQC):
                for m in range(NP):
                    base = 16 * (4 * qc + m)
                    if m >= 2 or qc > 0:
                        nqc, nm = (qc, m - 2) if m >= 2 else (qc - 1, m + 2)
                        if nqc < NQC - 1:
                            fill_at[base + 10].append(
                                lambda nqc=nqc, nm=nm: norm_task(nqc, nm))
                    if qc > 0 and m >= 2:
                        tt0 = 2 * (m - 2)
                        fill_at[base + 2].append(
                            lambda q=qc - 1, tt=tt0: wo_task(q, tt, 0))
                        fill_at[base + 5].append(
                            lambda q=qc - 1, tt=tt0: wo_task(q, tt, 1))
                        fill_at[base + 9].append(
                            lambda q=qc - 1, tt=tt0 + 1: wo_task(q, tt, 0))
                        fill_at[base + 12].append(
                            lambda q=qc - 1, tt=tt0 + 1: wo_task(q, tt, 1))
            # last q-chunk: per-pair reciprocal lets each norm release ~6
            # indices after its pair ends (norm(3,3) + all Wo(3,*) drain)
            for m in range(NP - 1):
                fill_at[16 * (4 * (NQC - 1) + m) + 22].append(
                    lambda nm=m: norm_task(NQC - 1, nm))

            # ---- prologue: minimum projections to start attention ----
            for step in proj_chunk_task(xq_sb, wq_sb, qt_sb, 0, 0):
                pass
            for step in proj_chunk_task(xk_sb, wk_sb, kt_sb, 0, 0):
                pass
            for step in v_chunk_task(0):
                pass

            # ---- main attention pipeline ----
            groups = [(qc, m, g) for qc in range(NQC) for m in range(NP)
                      for g in range(NG)]
            NGRP = len(groups)
            SKEW = 3
            pt_tiles = {}
            av_tiles = {}

            def emit_scores(gi):
                qc, m, g = groups[gi]
                qsl = slice(qc * QC, (qc + 1) * QC)
                ksl = slice(g * 128, (g + 1) * 128)
                s = sp.tile([128, 1024], F32, tag="s", name="s")
                nc.tensor.matmul(s[:, 0:512], kt_sb[0:64, m, ksl], qt_sb[0:64, m, qsl],
                                 start=True, stop=True, tile_position=(0, 0))
                nc.tensor.matmul(s[:, 512:1024], kt_sb[64:128, m, ksl], qt_sb[64:128, m, qsl],
                                 start=True, stop=True, tile_position=(64, 0))
                pt = ptp.tile([128, 1024], BF16, tag="pt", name="pt")
                pt_tiles[gi] = pt
                nc.scalar.activation(pt[:], s[:], EXP, scale=0.125)

            def emit_av(gi):
                qc, m, g = groups[gi]
                pt = pt_tiles.pop(gi)
                if g == 0:
                    av_tiles[(qc, m, 0)] = avp.tile([128, QC], F32, tag="av", name="avA")
                    av_tiles[(qc, m, 1)] = avp.tile([128, QC], F32, tag="av", name="avB")
                avA = av_tiles[(qc, m, 0)]
                avB = av_tiles[(qc, m, 1)]
                nc.tensor.matmul(avA[0:65, :], v_sb[:, g, 2 * m, 0:65], pt[:, 0:512],
                                 start=(g == 0), stop=(g == NG - 1))
                nc.tensor.matmul(avB[0:65, :], v_sb[:, g, 2 * m + 1, 0:65], pt[:, 512:1024],
                                 start=(g == 0), stop=(g == NG - 1))
                if g == NG - 1:
                    pair_end(qc, m)

            def pair_end(qc, m):
                avA = av_tiles.pop((qc, m, 0))
                avB = av_tiles.pop((qc, m, 1))
                av_sb = avsb.tile([128, 2 * QC], F32, tag="av_sb", name="av_sb")
                avsb_tiles[(qc, m)] = av_sb
                nc.vector.tensor_copy(av_sb[0:65, 0:QC], avA[0:65, :])
                nc.vector.tensor_copy(av_sb[0:65, QC:2 * QC], avB[0:65, :])
                # denominators: [1, 1024] -> [8, 128] repack keeps the
                # reciprocal's free size small (DVE divide is ~6.5 cyc/elem)
                den = denp.tile([8, 128], F32, tag="den", name="den")
                nc.sync.dma_start(den[:], av_sb[64:65, 0:2 * QC])
                recf = recp.tile([8, 128], F32, tag="recf", name="recf")
                nc.vector.reciprocal(recf[:], den[:])
                rec2 = recp.tile([8, 128], BF16, tag="rec2", name="rec2")
                nc.vector.tensor_copy(rec2[:], recf[:])
                rec_tiles[(qc, m)] = rec2

            av_cursor = 0
            for i in range(NGRP + SKEW):
                for f in fill_at.pop(i, []):
                    add_task(i, f())
                if i < NGRP:
                    emit_scores(i)
                if i < 20:
                    pump(i, 2)
                    while av_cursor <= min(i - SKEW, NGRP - 1):
                        emit_av(av_cursor)
                        av_cursor += 1
                else:
                    quota = 2 if i < NGRP else NGRP
                    while quota and av_cursor <= min(i - SKEW, NGRP - 1):
                        emit_av(av_cursor)
                        av_cursor += 1
                        quota -= 1
                    pump(i, 2)

            # ---- drain: remaining tasks + last pair's normalization + Wo ----
            pump(10 ** 9, 10 ** 9)
            # first two Wo chunks pre-accumulate fb 0..2 (they only need the
            # already-finished norms) while the last pair's reciprocal chain
            # runs; its norm uses a free scores-pool slot so the two open Wo
            # chunks can keep both misc banks
            g1 = wo_task(NQC - 1, 0, 0)
            g2 = wo_task(NQC - 1, 0, 1)
            for g in (g1, g2):
                for _ in range(3):
                    next(g)
            for _ in norm_task(NQC - 1, 3, pool=sp):
                pass
            for gen in [g1, g2] + [wo_task(NQC - 1, tt, jc)
                                   for tt in range(1, 4) for jc in range(2)]:
                for _ in gen:
                    pass

    nc.compile()
    return nc


def _get_nc():
    global _nc_cache
    if _nc_cache is None:
        _nc_cache = build()
    return _nc_cache


def kernel(query, key, value, W_q, W_k, W_v, W_o):
    global last_results
    nc = _get_nc()
    bf = ml_dtypes.bfloat16

    # broadcast masks for the [8, 128]-packed reciprocals: output column
    # block j picks row j (head A, partitions 0:64) / row 4+j (head B)
    mask = np.zeros((8, 512), bf)
    for j in range(4):
        mask[j, j * 128:j * 128 + 64] = 1.0
        mask[4 + j, j * 128 + 64:j * 128 + 128] = 1.0

    in_maps = []
    xt = {}
    for b in range(B):
        xt[b] = {
            "xq": np.ascontiguousarray(query[b].T).astype(bf),
            "xk": np.ascontiguousarray(key[b].T).astype(bf),
            "xv": np.ascontiguousarray(value[b].T).astype(bf),
        }
    wmaps = []
    for hg in range(2):
        r = slice(hg * FPC, (hg + 1) * FPC)
        wmaps.append({
            "wq": np.ascontiguousarray(W_q[r, :].T).astype(bf),
            "wk": np.ascontiguousarray(W_k[r, :].T).astype(bf),
            "wv": np.ascontiguousarray(W_v[r, :].T).astype(bf),
            "wo": np.ascontiguousarray(W_o[:, r].T).astype(bf),
        })
    for c in range(8):
        b, hg = c // 2, c % 2
        in_maps.append({**xt[b], **wmaps[hg], "mask": mask})

    res = run_bass_kernel_spmd(
        nc, in_maps, core_ids=list(range(8)),
        trace=bool(os.environ.get("BASS_KERNEL_TRACE")))
    last_results = res

    out = np.empty((B, S, D), np.float32)
    for b in range(B):
        out[b] = res.results[2 * b]["out"] + res.results[2 * b + 1]["out"]
    return out


# revision 41
# speedup vs baseline: 1.0110x; 1.0110x over previous
"""Multi-head attention (B=4, S=2048, D=1024, H=16, d_k=64) on 8 TRN2 NeuronCores.

Sharding: batch x head-group. Core c handles batch b = c//2 and heads
[8*(c%2), 8*(c%2)+8). Each core computes Q/K/V projections for its 512
output features (column-parallel), attention for its 8 heads, and a
row-parallel partial of the W_o output projection. The host sums the two
partials per batch (the row-parallel unshard) — no collectives needed.

Device schedule (per core): one software-pipelined stream. Attention is
split into 256 groups (4 q-chunks x 4 head-pairs x 16 key-blocks). Per
group the PE runs 2 score matmuls (N=512) and 2 attn@V matmuls; the ACT
engine runs one 1024-free exp per group and is the critical engine
(~280us), so everything else (Q/K/V projections, W_o, softmax
normalization) is emitted as filler inside the attention stream. attn@V
consumption is skewed 2 groups behind scores so the PE never head-waits
on the exp latency. PSUM: 3 rotating 2-bank score slots (also reused by
projection/W_o/normalization inserts) + 2 single-bank attn@V
accumulators (avA/avB of one pair; the skew covers their evacuation, so
the next pair reuses them without stalling). Softmax denominators ride
as a 65th V row; reciprocals are batched per half-q-chunk ([34, 512]
layout, pairs at partitions 0/32) on DVE; normalization and W_o are
deferred so their dependencies are always long ready.
"""

import os
from collections import defaultdict

import numpy as np
import ml_dtypes

import concourse.bacc as bacc
import concourse.mybir as mybir
import concourse.tile as tile
from concourse.bass_utils import run_bass_kernel_spmd

BF16 = mybir.dt.bfloat16
F32 = mybir.dt.float32
EXP = mybir.ActivationFunctionType.Exp

B, S, D = 4, 2048, 1024
H, DK = 16, 64
HPC = 8           # heads per core
FPC = HPC * DK    # 512 features per core
NP = 4            # head pairs per core
NB = 8            # din blocks of 128
NKB = 16          # key blocks of 128
QC = 512          # query chunk
NQC = S // QC     # 4
NG = NKB          # groups (1 key block each) per (qc, m)

_nc_cache = None
last_results = None


def build():
    nc = bacc.Bacc("TRN2", target_bir_lowering=False, debug=False, num_devices=8)

    xq = nc.dram_tensor("xq", [D, S], BF16, kind="ExternalInput").ap()
    xk = nc.dram_tensor("xk", [D, S], BF16, kind="ExternalInput").ap()
    xv = nc.dram_tensor("xv", [D, S], BF16, kind="ExternalInput").ap()
    wq = nc.dram_tensor("wq", [D, FPC], BF16, kind="ExternalInput").ap()
    wk = nc.dram_tensor("wk", [D, FPC], BF16, kind="ExternalInput").ap()
    wv = nc.dram_tensor("wv", [D, FPC], BF16, kind="ExternalInput").ap()
    wo = nc.dram_tensor("wo", [FPC, D], BF16, kind="ExternalInput").ap()
    mask = nc.dram_tensor("mask", [8, 512], BF16, kind="ExternalInput").ap()
    out = nc.dram_tensor("out", [S, D], F32, kind="ExternalOutput").ap()

    with tile.TileContext(nc) as tc:
        with (
            tc.tile_pool(name="wp", bufs=1) as wp,
            tc.tile_pool(name="qkv", bufs=1) as qkv,
            tc.tile_pool(name="xp", bufs=1) as xp,
            tc.tile_pool(name="xvp", bufs=3) as xvp,
            tc.tile_pool(name="ptp", bufs=4) as ptp,
            tc.tile_pool(name="avsb", bufs=2) as avsb,
            tc.tile_pool(name="otp", bufs=2) as otp,
            tc.tile_pool(name="denp", bufs=2) as denp,
            tc.tile_pool(name="recp", bufs=2) as recp,
            tc.tile_pool(name="outp", bufs=1) as outp,
            tc.tile_pool(name="sp", bufs=2, space="PSUM") as sp,
            tc.tile_pool(name="avp", bufs=2, space="PSUM") as avp,
            tc.tile_pool(name="miscp", bufs=2, space="PSUM") as miscp,
        ):
            # ---- static SBUF tensors + input DMAs (priority order) ----
            wq_sb = wp.tile([128, NB, NP, 128], BF16, tag="wq")
            wk_sb = wp.tile([128, NB, NP, 128], BF16, tag="wk")
            wv_sb = wp.tile([128, NB, FPC], BF16, tag="wv")
            wo_sb = wp.tile([128, NP, D], BF16, tag="wo")
            m_sb = wp.tile([8, NP, 128], BF16, tag="mask")
            qt_sb = qkv.tile([128, NP, S], BF16, tag="qt")
            kt_sb = qkv.tile([128, NP, S], BF16, tag="kt")
            v_sb = qkv.tile([128, NKB, HPC, 65], BF16, tag="v")
            xq_sb = xp.tile([128, NB, S], BF16, tag="xq")
            xk_sb = xp.tile([128, NB, S], BF16, tag="xk")

            # DMAs ordered by first consumption time. Each dma_start costs
            # ~600ns serially on the Sync sequencer, so bulk transfers use
            # few, larger descriptors (split only enough to spread queues);
            # the prologue-critical chunks are split 8-way for landing time.
            def dma_w(w_sb, w, m, nsplit=1):
                step = NB // nsplit
                for s in range(nsplit):
                    b0 = s * step
                    nc.sync.dma_start(
                        w_sb[:, b0:b0 + step, m],
                        w[:, m * 128:(m + 1) * 128]
                        .rearrange("(b p) c -> p b c", p=128)[:, b0:b0 + step])

            def dma_x(x_sb, x, c, nsplit=2):
                step = NB // nsplit
                for s in range(nsplit):
                    b0 = s * step
                    nc.sync.dma_start(
                        x_sb[:, b0:b0 + step, c * 512:(c + 1) * 512],
                        x[:, c * 512:(c + 1) * 512]
                        .rearrange("(b p) t -> p b t", p=128)[:, b0:b0 + step])

            xv_ch = {}
            def load_xv_chunk(c, nsplit=4):
                ch = xvp.tile([128, NB, 512], BF16, tag="xv", name=f"xv{c}")
                xv_ch[c] = ch
                step = NB // nsplit
                for s in range(nsplit):
                    b0 = s * step
                    nc.sync.dma_start(
                        ch[:, b0:b0 + step],
                        xv[:, c * 512:(c + 1) * 512]
                        .rearrange("(b p) t -> p b t", p=128)[:, b0:b0 + step])

            nc.sync.dma_start(m_sb[:], mask.rearrange("p (j c) -> p j c", c=128))
            dma_w(wq_sb, wq, 0, nsplit=2)
            dma_x(xq_sb, xq, 0, nsplit=8)
            dma_w(wk_sb, wk, 0, nsplit=2)
            dma_x(xk_sb, xk, 0, nsplit=8)
            for s in range(4):
                nc.sync.dma_start(
                    wv_sb[:, 2 * s:2 * s + 2],
                    wv.rearrange("(b p) c -> p b c", p=128)[:, 2 * s:2 * s + 2])
            load_xv_chunk(0)
            dma_x(xk_sb, xk, 1, nsplit=4)
            load_xv_chunk(1)
            dma_x(xk_sb, xk, 2, nsplit=4)
            dma_x(xk_sb, xk, 3, nsplit=4)
            dma_w(wq_sb, wq, 1)
            dma_w(wk_sb, wk, 1)
            load_xv_chunk(2)
            dma_w(wq_sb, wq, 2)
            dma_w(wk_sb, wk, 2)
            dma_w(wq_sb, wq, 3)
            dma_w(wk_sb, wk, 3)
            dma_x(xq_sb, xq, 1)
            for s in range(2):
                nc.sync.dma_start(
                    wo_sb[:, 2 * s:2 * s + 2],
                    wo.rearrange("(f p) c -> p f c", p=128)[:, 2 * s:2 * s + 2])
            dma_x(xq_sb, xq, 2)
            dma_x(xq_sb, xq, 3)

            nc.vector.memset(v_sb[:, :, :, 64], 1.0)

            # ---- filler tasks: generators yielding one PE matmul per step ----
            # Each task streams its accumulation through a 1-bank misc slot;
            # at most one task is open at a time (miscp bufs=2 lets the next
            # task's matmuls start while the previous slot's copy drains).
            def proj_chunk_task(x_sb, w_sb, dst, m, c):
                """512-token projection chunk: 8 matmuls + evac copy."""
                ps = miscp.tile([128, 512], F32, tag="misc", name="projps")
                lo = c * 512
                for b in range(NB):
                    nc.tensor.matmul(ps[:], w_sb[:, b, m], x_sb[:, b, lo:lo + 512],
                                     start=(b == 0), stop=(b == NB - 1))
                    if b < NB - 1:
                        yield
                nc.vector.tensor_copy(dst[:, m, lo:lo + 512], ps[:])

            def v_chunk_task(tt):
                """V projection for token tile tt: 8 matmuls + evac copy."""
                ps = miscp.tile([128, 512], F32, tag="misc", name="vps")
                ch = xv_ch[tt // 4]
                off = (tt % 4) * 128
                for b in range(NB):
                    nc.tensor.matmul(ps[:], ch[:, b, off:off + 128], wv_sb[:, b],
                                     start=(b == 0), stop=(b == NB - 1))
                    if b < NB - 1:
                        yield
                nc.vector.tensor_copy(
                    v_sb[:, tt, :, 0:64],
                    ps[:].rearrange("p (h c) -> p h c", c=64))

            ot_tiles = {}
            avsb_tiles = {}
            den_tiles = {}
            rec_tiles = {}

            def norm_task(qc, m):
                """Normalize pair (qc, m): broadcast 1/den via mask matmul, mul."""
                rec2 = rec_tiles[(qc, m)]
                if m == 0:
                    ot_tiles[qc] = otp.tile([128, NP, QC], BF16, tag="ot", name="ot")
                ot = ot_tiles[qc]
                av_sb = avsb_tiles.pop((qc, m))
                scp = miscp.tile([128, 512], F32, tag="misc", name="scp")
                for j in range(4):
                    nc.tensor.matmul(scp[:, 128 * j:128 * (j + 1)], m_sb[:, j],
                                     rec2[:], start=True, stop=True)
                nc.vector.tensor_mul(ot[0:64, m], av_sb[0:64, 0:QC], scp[0:64, 0:QC])
                nc.vector.tensor_mul(ot[64:128, m], av_sb[0:64, QC:2 * QC], scp[64:128, 0:QC])
                return
                yield

            def wo_task(qc, tt, jc):
                """Half of the output projection for token block (qc, tt)."""
                ot = ot_tiles[qc]
                wop = miscp.tile([128, 512], F32, tag="misc", name="wop")
                tsl = slice(tt * 128, (tt + 1) * 128)
                for fb in range(NP):
                    nc.tensor.matmul(
                        wop[:], ot[:, fb, tsl], wo_sb[:, fb, jc * 512:(jc + 1) * 512],
                        start=(fb == 0), stop=(fb == NP - 1))
                    if fb < NP - 1:
                        yield
                if jc == 0:
                    ostage_tiles[(qc, tt)] = outp.tile([128, D], F32, tag="ostage",
                                                       name="ostage")
                ostage = ostage_tiles[(qc, tt)]
                nc.vector.tensor_copy(ostage[:, jc * 512:(jc + 1) * 512], wop[:])
                row = qc * QC + tt * 128
                nc.sync.dma_start(out[row:row + 128, jc * 512:(jc + 1) * 512],
                                  ostage[:, jc * 512:(jc + 1) * 512])

            ostage_tiles = {}

            # Task queue: (due_index, generator). Pump keeps deadline order.
            tasks = []
            def add_task(due, gen):
                tasks.append((due, gen))
                tasks.sort(key=lambda t: t[0])

            open_task = None
            open_due = 0

            def pump(i, min_steps):
                """Emit filler matmul-steps: min_steps per index, running
                ahead only to finish overdue tasks (deadline-driven)."""
                nonlocal open_task, open_due
                emitted = 0
                while True:
                    if open_task is None and tasks and (
                            emitted < min_steps or tasks[0][0] <= i + 2):
                        open_due, open_task = tasks.pop(0)
                    if open_task is None:
                        return
                    if emitted >= min_steps and open_due > i:
                        return
                    try:
                        next(open_task)
                    except StopIteration:
                        open_task = None
                    emitted += 1

            # Due = first index whose scores/av READ the produced tile, minus
            # 3 so the pump's overdue-drain finishes the task strictly before
            # the consumer is emitted (PE program order is emission order).
            for tt in range(NKB):
                if tt >= 1:
                    add_task(2 + tt, v_chunk_task(tt))
            for m in range(NP):
                for c in range(4):
                    if not (m == 0 and c == 0):
                        add_task(16 * m + 4 * c - 3,
                                 proj_chunk_task(xk_sb, wk_sb, kt_sb, m, c))
            for m in range(NP):
                for c in range(4):
                    if not (m == 0 and c == 0):
                        add_task(64 * c + 16 * m - 3,
                                 proj_chunk_task(xq_sb, wq_sb, qt_sb, m, c))
            def load_xv_chunk_task(c):
                load_xv_chunk(c)
                return
                yield

            fill_at = defaultdict(list)
            fill_at[3].append(lambda: load_xv_chunk_task(3))
            # normalization / Wo release points: pair p=(qc, m) spans indices
            # [16p, 16p+16). norm of pair p-2 at +10 (reciprocal chain of p-2
            # finished ~5 indices earlier); Wo of q-chunk qc-1 in (qc, m2/m3).
            for qc in range(NQC):
                for m in range(NP):
                    base = 16 * (4 * qc + m)
                    if m >= 2 or qc > 0:
                        nqc, nm = (qc, m - 2) if m >= 2 else (qc - 1, m + 2)
                        if nqc < NQC - 1:
                            fill_at[base + 10].append(
                                lambda nqc=nqc, nm=nm: norm_task(nqc, nm))
                    if qc > 0 and m >= 2:
                        tt0 = 2 * (m - 2)
                        fill_at[base + 2].append(
                            lambda q=qc - 1, tt=tt0: wo_task(q, tt, 0))
                        fill_at[base + 5].append(
                            lambda q=qc - 1, tt=tt0: wo_task(q, tt, 1))
                        fill_at[base + 9].append(
                            lambda q=qc - 1, tt=tt0 + 1: wo_task(q, tt, 0))
                        fill_at[base + 12].append(
                            lambda q=qc - 1, tt=tt0 + 1: wo_task(q, tt, 1))
            # last q-chunk: per-pair reciprocal lets each norm release ~6
            # indices after its pair ends (norm(3,3) + all Wo(3,*) drain)
            for m in range(NP - 1):
                fill_at[16 * (4 * (NQC - 1) + m) + 22].append(
                    lambda nm=m: norm_task(NQC - 1, nm))

            # ---- prologue: minimum projections to start attention ----
            for step in proj_chunk_task(xq_sb, wq_sb, qt_sb, 0, 0):
                pass
            for step in proj_chunk_task(xk_sb, wk_sb, kt_sb, 0, 0):
                pass
            for step in v_chunk_task(0):
                pass

            # ---- main attention pipeline ----
            groups = [(qc, m, g) for qc in range(NQC) for m in range(NP)
                      for g in range(NG)]
            NGRP = len(groups)
            SKEW = 3
            pt_tiles = {}
            av_tiles = {}

            def emit_scores(gi):
                qc, m, g = groups[gi]
                qsl = slice(qc * QC, (qc + 1) * QC)
                ksl = slice(g * 128, (g + 1) * 128)
                s = sp.tile([128, 1024], F32, tag="s", name="s")
                nc.tensor.matmul(s[:, 0:512], kt_sb[0:64, m, ksl], qt_sb[0:64, m, qsl],
                                 start=True, stop=True, tile_position=(0, 0))
                nc.tensor.matmul(s[:, 512:1024], kt_sb[64:128, m, ksl], qt_sb[64:128, m, qsl],
                                 start=True, stop=True, tile_position=(64, 0))
                pt = ptp.tile([128, 1024], BF16, tag="pt", name="pt")
                pt_tiles[gi] = pt
                nc.scalar.activation(pt[:], s[:], EXP, scale=0.125)

            def emit_av(gi):
                qc, m, g = groups[gi]
                pt = pt_tiles.pop(gi)
                if g == 0:
                    av_tiles[(qc, m, 0)] = avp.tile([128, QC], F32, tag="av", name="avA")
                    av_tiles[(qc, m, 1)] = avp.tile([128, QC], F32, tag="av", name="avB")
                avA = av_tiles[(qc, m, 0)]
                avB = av_tiles[(qc, m, 1)]
                nc.tensor.matmul(avA[0:65, :], v_sb[:, g, 2 * m, 0:65], pt[:, 0:512],
                                 start=(g == 0), stop=(g == NG - 1))
                nc.tensor.matmul(avB[0:65, :], v_sb[:, g, 2 * m + 1, 0:65], pt[:, 512:1024],
                                 start=(g == 0), stop=(g == NG - 1))
                if g == NG - 1:
                    pair_end(qc, m)

            def pair_end(qc, m):
                avA = av_tiles.pop((qc, m, 0))
                avB = av_tiles.pop((qc, m, 1))
                av_sb = avsb.tile([128, 2 * QC], F32, tag="av_sb", name="av_sb")
                avsb_tiles[(qc, m)] = av_sb
                nc.vector.tensor_copy(av_sb[0:65, 0:QC], avA[0:65, :])
                nc.vector.tensor_copy(av_sb[0:65, QC:2 * QC], avB[0:65, :])
                # denominators: [1, 1024] -> [8, 128] repack keeps the
                # reciprocal's free size small (DVE divide is ~6.5 cyc/elem)
                den = denp.tile([8, 128], F32, tag="den", name="den")
                nc.sync.dma_start(den[:], av_sb[64:65, 0:2 * QC])
                recf = recp.tile([8, 128], F32, tag="recf", name="recf")
                nc.vector.reciprocal(recf[:], den[:])
                rec2 = recp.tile([8, 128], BF16, tag="rec2", name="rec2")
                nc.vector.tensor_copy(rec2[:], recf[:])
                rec_tiles[(qc, m)] = rec2

            av_cursor = 0
            for i in range(NGRP + SKEW):
                for f in fill_at.pop(i, []):
                    add_task(i, f())
                if i < NGRP:
                    emit_scores(i)
                if i < 20:
                    pump(i, 2)
                    while av_cursor <= min(i - SKEW, NGRP - 1):
                        emit_av(av_cursor)
                        av_cursor += 1
                else:
                    quota = 2 if i < NGRP else NGRP
                    while quota and av_cursor <= min(i - SKEW, NGRP - 1):
                        emit_av(av_cursor)
                        av_cursor += 1
                        quota -= 1
                    pump(i, 2)

            # ---- drain: remaining tasks + last pair's normalization + Wo ----
            pump(10 ** 9, 10 ** 9)
            for gen in ([norm_task(NQC - 1, 3)] +
                        [wo_task(NQC - 1, tt, jc) for tt in range(4) for jc in range(2)]):
                for _ in gen:
                    pass

    nc.compile()
    return nc


def _get_nc():
    global _nc_cache
    if _nc_cache is None:
        _nc_cache = build()
    return _nc_cache


def kernel(query, key, value, W_q, W_k, W_v, W_o):
    global last_results
    nc = _get_nc()
    bf = ml_dtypes.bfloat16

    # broadcast masks for the [8, 128]-packed reciprocals: output column
    # block j picks row j (head A, partitions 0:64) / row 4+j (head B)
    mask = np.zeros((8, 512), bf)
    for j in range(4):
        mask[j, j * 128:j * 128 + 64] = 1.0
        mask[4 + j, j * 128 + 64:j * 128 + 128] = 1.0

    in_maps = []
    xt = {}
    for b in range(B):
        xt[b] = {
            "xq": np.ascontiguousarray(query[b].T).astype(bf),
            "xk": np.ascontiguousarray(key[b].T).astype(bf),
            "xv": np.ascontiguousarray(value[b].T).astype(bf),
        }
    wmaps = []
    for hg in range(2):
        r = slice(hg * FPC, (hg + 1) * FPC)
        wmaps.append({
            "wq": np.ascontiguousarray(W_q[r, :].T).astype(bf),
            "wk": np.ascontiguousarray(W_k[r, :].T).astype(bf),
            "wv": np.ascontiguousarray(W_v[r, :].T).astype(bf),
            "wo": np.ascontiguousarray(W_o[:, r].T).astype(bf),
        })
    for c in range(8):
        b, hg = c // 2, c % 2
        in_maps.append({**xt[b], **wmaps[hg], "mask": mask})

    res = run_bass_kernel_spmd(
        nc, in_maps, core_ids=list(range(8)),
        trace=bool(os.environ.get("BASS_KERNEL_TRACE")))
    last_results = res

    out = np.empty((B, S, D), np.float32)
    for b in range(B):
        out[b] = res.results[2 * b]["out"] + res.results[2 * b + 1]["out"]
    return out
